# revision 33
# baseline (speedup 1.0000x reference)
"""Causal self-attention on 8 trn2 NeuronCores.

Sharding: core c -> (batch b = c//2, head-group g = c%2).  Each head-group
is 8 heads = 512 channels.  Per core, one flat pipeline of per-(window,
head-pair) attention rounds with projection / out-projection groups
interleaved into the tensor queue's dependency-stall slots.

  - x and all weights are pre-swizzled on the host into their SBUF
    layouts, so every input DMA is contiguous; launches are split across
    the two HWDGE queues (sync + scalar) and ordered so round (0,0) only
    waits for x(w0) + Wq(j0) + Wk(j0) + Wv.  A burst of warmup matmuls
    on a zeroed scratch tile runs during the DMA wait so the PE HAM
    clock-gate is released before the first real matmul.
  - q/k/v projections in bf16 (8 contraction chunks of 128).  Their
    PSUM->SBUF epilogues run on ScalarE for the early (vector-congested)
    windows and on VectorE later (scalar-congested by exp).
  - QK^T as two concurrent row-tiled K=64 matmuls (heads 2j / 2j+1 live
    in partitions 0-63 / 64-127 of qT/kT); outputs land in separate PSUM
    banks of one [P, 2, W] tile.
  - exp on ScalarE into bf16 tiles [P, a, W], one per tk-tile, sized to
    the tile's valid (causal) query range.
  - AV accumulates per tk-tile into a [65, W] PSUM bank per head; a
    ones-column per head in vp yields the softmax denominators for free.
  - causal masking multiplies each diagonal tile's [P, 2, P] block by a
    bf16 triangular mask (both heads in one DVE op).
  - normalization: one [65, W] high-priority copy releases PSUM, then
    reciprocal + gpsimd partition-broadcast + multiply into yT.
  - output projection bf16, stored f16; all of it is interleaved into
    the scalar-bound last window, except half of oproj(2) + oproj(3)
    which run behind the last attention round (alternating between two
    PSUM pools and both HWDGE queues) so the PE stays warm through the
    final normalization chain.  Host sums the two partials per batch and
    adds (bv @ Wo + bo).
"""

import numpy as np
import ml_dtypes

import concourse.bass as bass
import concourse.mybir as mybir
from concourse import bacc, tile
from concourse.bass_utils import run_bass_kernel_spmd

B, T, C, H = 4, 2048, 1024, 16
HD = C // H          # 64
G = 2                # head groups (cores per batch)
HG = H // G          # 8 heads per group
CG = C // G          # 512 channels per group
CGP = CG // 128      # 4 c_out tiles per group
P = 128
W = 512              # free-dim window (one PSUM bank of f32)
NW = T // W          # 4 windows
NTT = T // P         # 16 t tiles
NCI = C // P         # 8 c_in chunks
VS = HD + 1          # 65: v plus ones column

_cached_nc = None


def _build():
    f32 = mybir.dt.float32
    f16 = mybir.dt.float16
    bf16 = mybir.dt.bfloat16
    AF = mybir.ActivationFunctionType
    nc = bacc.Bacc("TRN2", target_bir_lowering=False, debug=False, num_devices=8)

    # x pre-swizzled to window-major [P, w, chunk, t] (contiguous DMAs)
    xt_d = nc.dram_tensor("xt", [P, NW * NCI * W], bf16, kind="ExternalInput")
    # weights pre-swizzled on host to the "p c n" SBUF layout
    wq_d = nc.dram_tensor("wq", [P, NCI * CG], bf16, kind="ExternalInput")
    wk_d = nc.dram_tensor("wk", [P, NCI * CG], bf16, kind="ExternalInput")
    wv_d = nc.dram_tensor("wv", [P, NCI * CG], bf16, kind="ExternalInput")
    wo_d = nc.dram_tensor("wo", [P, CGP * C], bf16, kind="ExternalInput")
    bqk_d = nc.dram_tensor("bqk", [P, 2 * CGP], f32, kind="ExternalInput")
    mask_d = nc.dram_tensor("maskT", [P, 2 * P], bf16, kind="ExternalInput")
    out_d = nc.dram_tensor("outp", [C, T], f16, kind="ExternalOutput")

    def mm(out, lhsT, rhs, start, stop, **kw):
        return nc.tensor.matmul(out, lhsT, rhs, start=start, stop=stop, **kw)

    escale = 1.0 / float(np.sqrt(HD))

    with tile.TileContext(nc) as tc:
        with tc.tile_pool(name="pers", bufs=1) as pers:
            qT = pers.tile([P, CGP, T], bf16)
            kT = pers.tile([P, CGP, T], bf16)
            yT = pers.tile([P, CGP, T], bf16)
            vp = pers.tile([P, NTT, HG * VS], bf16)
            wo_sb = pers.tile([P, CGP, C], bf16)
            whqk = pers.tile([P, 2, CGP, NCI, P], bf16)
            whv = pers.tile([P, NCI, CG], bf16)
            xc = pers.tile([P, NW, NCI, W], bf16)
            bqk_sb = pers.tile([P, 2, CGP], f32)
            maskT2 = pers.tile([P, 2, P], bf16)
            scr = pers.tile([P, W], bf16)

            # ones columns of vp (v writes fill the other lanes)
            ones_lanes = vp.rearrange("p t (h x) -> p t h x", x=VS)[:, :, :, HD:VS]
            nc.vector.memset(ones_lanes, 1.0)
            nc.vector.memset(scr, 0.0)

            # ---- DMAs: two HWDGE queues, in the order compute needs ----
            xv = xt_d.ap().rearrange("p (w c t) -> p w c t", w=NW, c=NCI)
            wqv = wq_d.ap().rearrange("p (j c n) -> p j c n", j=CGP, c=NCI)
            wkv = wk_d.ap().rearrange("p (j c n) -> p j c n", j=CGP, c=NCI)
            # sync queue: Wq j0, x(w0), remaining Wq/Wk j-blocks, then x
            nc.sync.dma_start(out=whqk[:, 0, 0], in_=wqv[:, 0])
            nc.sync.dma_start(out=xc[:, 0, 0:2], in_=xv[:, 0, 0:2])
            nc.sync.dma_start(out=xc[:, 0, 2:8], in_=xv[:, 0, 2:8])
            for j in range(1, CGP):
                nc.sync.dma_start(out=whqk[:, 0, j], in_=wqv[:, j])
                nc.sync.dma_start(out=whqk[:, 1, j], in_=wkv[:, j])
            for w in range(1, NW):
                nc.sync.dma_start(out=xc[:, w], in_=xv[:, w])
            # scalar queue (idle until the first exp): biases, mask, Wk0, Wv, Wo
            nc.scalar.dma_start(
                out=bqk_sb,
                in_=bqk_d.ap().rearrange("p (b j) -> p b j", b=2))
            nc.scalar.dma_start(
                out=maskT2,
                in_=mask_d.ap().rearrange("p (a q) -> p a q", a=2))
            nc.scalar.dma_start(out=whqk[:, 1, 0], in_=wkv[:, 0])
            nc.scalar.dma_start(
                out=whv,
                in_=wv_d.ap().rearrange("p (c n) -> p c n", c=NCI))
            nc.scalar.dma_start(
                out=wo_sb,
                in_=wo_d.ap().rearrange("p (c n) -> p c n", c=CGP))

            with (
                tc.tile_pool(name="ppa", bufs=2, space="PSUM") as ppa,
                tc.tile_pool(name="pqk", bufs=2, space="PSUM") as pqk,
                tc.tile_pool(name="pav", bufs=2, space="PSUM") as pav,
                tc.tile_pool(name="ptp", bufs=12) as ptp,
                tc.tile_pool(name="avs", bufs=6) as avsp,
                tc.tile_pool(name="dnp", bufs=6) as dnp,
                tc.tile_pool(name="rbp", bufs=6) as rbp,
                tc.tile_pool(name="otp", bufs=6) as otp,
            ):
                # warm up the PE (HAM clock-gate) while input DMAs stream
                wps = pav.tile([P, W], f32, tag="av", name="warm")
                for _ in range(16):
                    mm(wps, scr[:, 0:P], scr, start=True, stop=True)

                def proj_groups(w, sc_a, sc_b):
                    """(A, B) closure lists, each one psum round.

                    A: needed before round (w, 0) -- q/k j0 and the v
                    tiles.  B: q/k j1..3, interleaved into window w
                    itself ahead of the rounds that read them.  sc_a /
                    sc_b: run the PSUM->SBUF epilogue on ScalarE (early
                    windows, where VectorE is the congested engine)."""

                    def qk_group(wi, dst, j, sc):
                        def g():
                            ps = ppa.tile([P, W], f32, tag="pp", name="psqj")
                            for i in range(NCI):
                                mm(ps, whqk[:, wi, j, i, :],
                                   xc[:, w, i, :],
                                   start=(i == 0), stop=(i == NCI - 1))
                            if sc:
                                nc.scalar.activation(
                                    dst[:, j, w * W:(w + 1) * W], ps,
                                    AF.Identity, bias=bqk_sb[:, wi, j:j + 1])
                            else:
                                nc.vector.tensor_scalar_add(
                                    dst[:, j, w * W:(w + 1) * W], ps,
                                    bqk_sb[:, wi, j:j + 1])
                        return g

                    def v_group(it, sc):
                        def g():
                            ps = ppa.tile([P, W], f32, tag="pp", name="psvt")
                            for i in range(NCI):
                                mm(ps, xc[:, w, i, (it % 4) * P:
                                          (it % 4 + 1) * P],
                                   whv[:, i, :],
                                   start=(i == 0), stop=(i == NCI - 1))
                            vdst = (vp[:, it, :]
                                    .rearrange("p (h x) -> p h x",
                                               x=VS)[:, :, 0:HD])
                            psh = ps.rearrange("p (h x) -> p h x", x=HD)
                            if sc:
                                nc.scalar.copy(vdst, psh)
                            else:
                                nc.vector.tensor_copy(vdst, psh)
                        return g

                    ga = [qk_group(0, qT, 0, sc_a), qk_group(1, kT, 0, sc_a)]
                    ga += [v_group(it, sc_a) for it in range(4 * w, 4 * w + 4)]
                    gb = []
                    for j in range(1, CGP):
                        gb.append(qk_group(0, qT, j, sc_b))
                        gb.append(qk_group(1, kT, j, sc_b))
                    return ga, gb

                def oproj_groups(w, tail=False):
                    ws = slice(w * W, (w + 1) * W)

                    def o_group(m):
                        def g():
                            pool = pav if (tail and m % 2 == 1) else ppa
                            tag = "av" if pool is pav else "pp"
                            po = pool.tile([P, W], f32, tag=tag, name="po")
                            for i in range(CGP):
                                mm(po, wo_sb[:, i, m * P:(m + 1) * P],
                                   yT[:, i, ws],
                                   start=(i == 0), stop=(i == CGP - 1))
                            ot = otp.tile([P, W], f16, tag="ot")
                            if tail and m % 2 == 1:
                                nc.scalar.copy(ot, po)
                            else:
                                nc.vector.tensor_copy(ot, po)
                            dq = nc.scalar if (tail and m % 2 == 1) else nc.sync
                            dq.dma_start(
                                out=out_d.ap()[m * P:(m + 1) * P, ws],
                                in_=ot)
                        return g

                    return [o_group(m) for m in range(C // P)]

                # prologue: only what round (0,0) strictly needs
                ga0, gb0 = proj_groups(0, False, False)
                for g in ga0:
                    g()

                defer = gb0   # B-groups run inside their own window
                tail_work = None
                sc_flags = {1: (True, False), 2: (False, False),
                            3: (False, False)}
                for w in range(NW):
                    ws = slice(w * W, (w + 1) * W)
                    tq0 = w * W
                    ntk = 4 * (w + 1)

                    work = list(defer)
                    defer = []
                    if w == NW - 1:
                        # all out-projections land here: windows 0-2 are
                        # tensor-bound, window 3 is scalar-bound (exp), so
                        # oproj work fills w3's tensor idle.  Half of
                        # oproj(2) stays behind the last attention round
                        # to cover the final normalization chain (PE
                        # stays warm into oproj(3)).
                        og2 = oproj_groups(NW - 2, tail=True)
                        work += oproj_groups(0) + oproj_groups(1) + og2[:4]
                        tail_work = og2[4:]
                    if w < NW - 1:
                        ga, gb = proj_groups(w + 1, *sc_flags[w + 1])
                        work += ga
                        defer = gb
                    nwork = len(work)
                    total_tiles = CGP * ntk
                    seen = 0
                    emitted = 0

                    for j in range(CGP):
                        # round boundary: flush all uniformly-due work so
                        # chunked emission can never defer a q/k j-block
                        # past the round that reads it
                        while emitted < (nwork * seen) // total_tiles:
                            work[emitted]()
                            emitted += 1
                        ps_av = [pav.tile([VS, W], f32, tag="av",
                                          name=f"av{w}_{j}_{a}")
                                 for a in range(2)]
                        for p in range(ntk // 2):
                            # two tiles per batch: QK,QK then AV,AV,AV,AV.
                            # Full-row LDWEIGHTS hide inside same-shape mm
                            # chains but are exposed at every QK<->AV
                            # transition; pairing halves the transitions.
                            pair = (2 * p, 2 * p + 1)
                            c0s, pts = [], []
                            for i in pair:
                                nn = tq0 + W - max(tq0, i * P)
                                c0 = W - nn
                                c0s.append(c0)
                                psqk = pqk.tile([P, 2, W], f32, tag="qk")
                                for a in range(2):
                                    mm(psqk[:, a, c0:],
                                       kT[64 * a:64 * (a + 1), j,
                                          i * P:(i + 1) * P],
                                       qT[64 * a:64 * (a + 1), j,
                                          tq0 + c0:tq0 + W],
                                       start=True, stop=True)
                                seen += 1
                                # emit interleave work two groups at a
                                # time: halves the proj<->attention
                                # transitions (each pays an exposed
                                # LDWEIGHTS on the PE)
                                target = (nwork * seen) // total_tiles
                                if target - emitted >= 3 or seen == total_tiles:
                                    while emitted < target:
                                        work[emitted]()
                                        emitted += 1
                                pt_t = ptp.tile([P, 2, W], bf16, tag="pt")
                                pts.append(pt_t)
                                nc.scalar.activation(
                                    pt_t[:, :, c0:], psqk[:, :, c0:],
                                    AF.Exp, scale=escale)
                                if i * P >= tq0:  # diagonal: mask its block
                                    with tc.high_priority():
                                        nc.vector.tensor_mul(
                                            pt_t[:, :, c0:c0 + P],
                                            pt_t[:, :, c0:c0 + P], maskT2)
                            for ti, i in enumerate(pair):
                                c0 = c0s[ti]
                                for a in range(2):
                                    h = 2 * j + a
                                    mm(ps_av[a][:, c0:],
                                       vp[:, i, h * VS:(h + 1) * VS],
                                       pts[ti][:, a, c0:],
                                       start=(i == 0), stop=(i == ntk - 1))
                        for a in range(2):
                            av_sb = avsp.tile([VS, W], f32, tag="avs")
                            with tc.high_priority():
                                nc.vector.tensor_copy(av_sb, ps_av[a])
                            dn = dnp.tile([1, W], f32, tag="dn")
                            nc.vector.tensor_copy(dn, av_sb[HD:VS, :])
                            nc.vector.reciprocal_approx_fast(
                                out=dn, in_=dn)
                            rb = rbp.tile([HD, W], f32, tag="rb")
                            nc.gpsimd.partition_broadcast(rb, dn)
                            nc.vector.tensor_mul(
                                yT[64 * a:64 * (a + 1), j, ws],
                                av_sb[0:HD, :], rb)
                    while emitted < nwork:
                        work[emitted]()
                        emitted += 1

                for g in tail_work + oproj_groups(NW - 1, tail=True):
                    g()

    nc.compile()
    return nc


def get_nc():
    global _cached_nc
    if _cached_nc is None:
        _cached_nc = _build()
    return _cached_nc


def _swz(w, nci):
    """[K, N] -> contiguous [P, nci*N] in the 'p c n' SBUF layout."""
    k, n = w.shape
    return np.ascontiguousarray(
        w.reshape(nci, P, n).transpose(1, 0, 2).reshape(P, nci * n)
        .astype(ml_dtypes.bfloat16))


def _swzj(w):
    """[C, CG] -> contiguous [P, CGP*NCI*P] in the j-major SBUF layout."""
    return np.ascontiguousarray(
        w.reshape(NCI, P, CGP, P).transpose(1, 2, 0, 3)
        .reshape(P, CGP * NCI * P).astype(ml_dtypes.bfloat16))


def make_in_maps(x, Wq, bq, Wk, bk, Wv, bv, Wo, bo):
    x = np.asarray(x, np.float32)
    tri = np.triu(np.ones((P, P), np.float32))
    tri2 = np.concatenate([tri, tri], axis=1)
    in_maps = []
    for c in range(8):
        b, g = c // 2, c % 2
        cs = slice(g * CG, (g + 1) * CG)
        bqk = np.concatenate([
            np.asarray(bq, np.float32)[cs].reshape(CGP, P).T,
            np.asarray(bk, np.float32)[cs].reshape(CGP, P).T], axis=1)
        # x[b].T is [C, T]; window-major swizzle to [P, w, chunk, t]
        xt = x[b].T.astype(ml_dtypes.bfloat16)
        xt = (xt.reshape(NCI, P, NW, W).transpose(1, 2, 0, 3)
              .reshape(P, NW * NCI * W))
        in_maps.append({
            "xt": np.ascontiguousarray(xt),
            "wq": _swzj(np.asarray(Wq, np.float32)[:, cs]),
            "wk": _swzj(np.asarray(Wk, np.float32)[:, cs]),
            "wv": _swz(np.asarray(Wv, np.float32)[:, cs], NCI),
            "wo": _swz(np.asarray(Wo, np.float32)[cs, :], CGP),
            "bqk": np.ascontiguousarray(bqk),
            "maskT": tri2.astype(ml_dtypes.bfloat16),
        })
    return in_maps


def combine(results, Wv, bv, Wo, bo):
    const = (np.asarray(bv, np.float32) @ np.asarray(Wo, np.float32)
             + np.asarray(bo, np.float32))
    out = np.empty((B, T, C), np.float32)
    for b in range(B):
        acc = (results[2 * b]["outp"].astype(np.float32)
               + results[2 * b + 1]["outp"].astype(np.float32))
        out[b] = acc.T + const[None, :]
    return out


def kernel(x, Wq, bq, Wk, bk, Wv, bv, Wo, bo):
    nc = get_nc()
    in_maps = make_in_maps(x, Wq, bq, Wk, bk, Wv, bv, Wo, bo)
    res = run_bass_kernel_spmd(nc, in_maps, core_ids=list(range(8)))
    return combine(res.results, Wv, bv, Wo, bo)


# revision 34
# speedup vs baseline: 1.0064x; 1.0064x over previous
"""Causal self-attention on 8 trn2 NeuronCores.

Sharding: core c -> (batch b = c//2, head-group g = c%2).  Each head-group
is 8 heads = 512 channels.  Per core, one flat pipeline of per-(window,
head-pair) attention rounds with projection / out-projection groups
interleaved into the tensor queue's dependency-stall slots.

  - x and all weights are pre-swizzled on the host into their SBUF
    layouts, so every input DMA is contiguous; launches are split across
    the two HWDGE queues (sync + scalar) and ordered so round (0,0) only
    waits for x(w0) + Wq(j0) + Wk(j0) + Wv.  A burst of warmup matmuls
    on a zeroed scratch tile runs during the DMA wait so the PE HAM
    clock-gate is released before the first real matmul.
  - q/k/v projections in bf16 (8 contraction chunks of 128).  Their
    PSUM->SBUF epilogues run on ScalarE for the early (vector-congested)
    windows and on VectorE later (scalar-congested by exp).
  - QK^T as two concurrent row-tiled K=64 matmuls (heads 2j / 2j+1 live
    in partitions 0-63 / 64-127 of qT/kT); outputs land in separate PSUM
    banks of one [P, 2, W] tile.
  - exp on ScalarE into bf16 tiles [P, a, W], one per tk-tile, sized to
    the tile's valid (causal) query range.
  - AV accumulates per tk-tile into a [65, W] PSUM bank per head; a
    ones-column per head in vp yields the softmax denominators for free.
  - causal masking multiplies each diagonal tile's [P, 2, P] block by a
    bf16 triangular mask (both heads in one DVE op).
  - normalization: one [65, W] high-priority copy releases PSUM, then
    reciprocal + gpsimd partition-broadcast + multiply into yT.
  - output projection bf16, stored f16; all of it is interleaved into
    the scalar-bound last window, except half of oproj(2) + oproj(3)
    which run behind the last attention round (alternating between two
    PSUM pools and both HWDGE queues) so the PE stays warm through the
    final normalization chain.  Host sums the two partials per batch and
    adds (bv @ Wo + bo).
"""

import numpy as np
import ml_dtypes

import concourse.bass as bass
import concourse.mybir as mybir
from concourse import bacc, tile
from concourse.bass_utils import run_bass_kernel_spmd

B, T, C, H = 4, 2048, 1024, 16
HD = C // H          # 64
G = 2                # head groups (cores per batch)
HG = H // G          # 8 heads per group
CG = C // G          # 512 channels per group
CGP = CG // 128      # 4 c_out tiles per group
P = 128
W = 512              # free-dim window (one PSUM bank of f32)
NW = T // W          # 4 windows
NTT = T // P         # 16 t tiles
NCI = C // P         # 8 c_in chunks
VS = HD + 1          # 65: v plus ones column

_cached_nc = None


def _build():
    f32 = mybir.dt.float32
    f16 = mybir.dt.float16
    bf16 = mybir.dt.bfloat16
    AF = mybir.ActivationFunctionType
    nc = bacc.Bacc("TRN2", target_bir_lowering=False, debug=False, num_devices=8)

    # x pre-swizzled to window-major [P, w, chunk, t] (contiguous DMAs)
    xt_d = nc.dram_tensor("xt", [P, NW * NCI * W], bf16, kind="ExternalInput")
    # weights pre-swizzled on host to the "p c n" SBUF layout
    wq_d = nc.dram_tensor("wq", [P, NCI * CG], bf16, kind="ExternalInput")
    wk_d = nc.dram_tensor("wk", [P, NCI * CG], bf16, kind="ExternalInput")
    wv_d = nc.dram_tensor("wv", [P, NCI * CG], bf16, kind="ExternalInput")
    wo_d = nc.dram_tensor("wo", [P, CGP * C], bf16, kind="ExternalInput")
    bqk_d = nc.dram_tensor("bqk", [P, 2 * CGP], f32, kind="ExternalInput")
    mask_d = nc.dram_tensor("maskT", [P, 2 * P], bf16, kind="ExternalInput")
    out_d = nc.dram_tensor("outp", [C, T], f16, kind="ExternalOutput")

    def mm(out, lhsT, rhs, start, stop, **kw):
        return nc.tensor.matmul(out, lhsT, rhs, start=start, stop=stop, **kw)

    escale = 1.0 / float(np.sqrt(HD))

    with tile.TileContext(nc) as tc:
        with tc.tile_pool(name="pers", bufs=1) as pers:
            qT = pers.tile([P, CGP, T], bf16)
            kT = pers.tile([P, CGP, T], bf16)
            yT = pers.tile([P, CGP, T], bf16)
            vp = pers.tile([P, NTT, HG * VS], bf16)
            wo_sb = pers.tile([P, CGP, C], bf16)
            whqk = pers.tile([P, 2, CGP, NCI, P], bf16)
            whv = pers.tile([P, NCI, CG], bf16)
            xc = pers.tile([P, NW, NCI, W], bf16)
            bqk_sb = pers.tile([P, 2, CGP], f32)
            maskT2 = pers.tile([P, 2, P], bf16)
            scr = pers.tile([P, W], bf16)

            # ones columns of vp (v writes fill the other lanes)
            ones_lanes = vp.rearrange("p t (h x) -> p t h x", x=VS)[:, :, :, HD:VS]
            nc.vector.memset(ones_lanes, 1.0)
            nc.vector.memset(scr, 0.0)

            # ---- DMAs: two HWDGE queues, in the order compute needs ----
            xv = xt_d.ap().rearrange("p (w c t) -> p w c t", w=NW, c=NCI)
            wqv = wq_d.ap().rearrange("p (j c n) -> p j c n", j=CGP, c=NCI)
            wkv = wk_d.ap().rearrange("p (j c n) -> p j c n", j=CGP, c=NCI)
            # sync queue: Wq j0, x(w0), remaining Wq/Wk j-blocks, then x
            nc.sync.dma_start(out=whqk[:, 0, 0], in_=wqv[:, 0])
            nc.sync.dma_start(out=xc[:, 0, 0:2], in_=xv[:, 0, 0:2])
            nc.sync.dma_start(out=xc[:, 0, 2:8], in_=xv[:, 0, 2:8])
            for j in range(1, CGP):
                nc.sync.dma_start(out=whqk[:, 0, j], in_=wqv[:, j])
                nc.sync.dma_start(out=whqk[:, 1, j], in_=wkv[:, j])
            for w in range(1, NW):
                nc.sync.dma_start(out=xc[:, w], in_=xv[:, w])
            # scalar queue (idle until the first exp): biases, mask, Wk0, Wv, Wo
            nc.scalar.dma_start(
                out=bqk_sb,
                in_=bqk_d.ap().rearrange("p (b j) -> p b j", b=2))
            nc.scalar.dma_start(
                out=maskT2,
                in_=mask_d.ap().rearrange("p (a q) -> p a q", a=2))
            nc.scalar.dma_start(out=whqk[:, 1, 0], in_=wkv[:, 0])
            nc.scalar.dma_start(
                out=whv,
                in_=wv_d.ap().rearrange("p (c n) -> p c n", c=NCI))
            nc.scalar.dma_start(
                out=wo_sb,
                in_=wo_d.ap().rearrange("p (c n) -> p c n", c=CGP))

            with (
                tc.tile_pool(name="ppa", bufs=2, space="PSUM") as ppa,
                tc.tile_pool(name="pqk", bufs=2, space="PSUM") as pqk,
                tc.tile_pool(name="pav", bufs=2, space="PSUM") as pav,
                tc.tile_pool(name="ptp", bufs=12) as ptp,
                tc.tile_pool(name="avs", bufs=4) as avsp,
                tc.tile_pool(name="dnp", bufs=4) as dnp,
                tc.tile_pool(name="rbp", bufs=4) as rbp,
                tc.tile_pool(name="otp", bufs=6) as otp,
            ):
                # warm up the PE (HAM clock-gate) while input DMAs stream
                wps = pav.tile([P, W], f32, tag="av", name="warm")
                for _ in range(16):
                    mm(wps, scr[:, 0:P], scr, start=True, stop=True)

                def proj_groups(w, sc_a, sc_b):
                    """(A, B) closure lists, each one psum round.

                    A: needed before round (w, 0) -- q/k j0 and the v
                    tiles.  B: q/k j1..3, interleaved into window w
                    itself ahead of the rounds that read them.  sc_a /
                    sc_b: run the PSUM->SBUF epilogue on ScalarE (early
                    windows, where VectorE is the congested engine)."""

                    def qk_group(wi, dst, j, sc):
                        def g():
                            ps = ppa.tile([P, W], f32, tag="pp", name="psqj")
                            for i in range(NCI):
                                mm(ps, whqk[:, wi, j, i, :],
                                   xc[:, w, i, :],
                                   start=(i == 0), stop=(i == NCI - 1))
                            if sc:
                                nc.scalar.activation(
                                    dst[:, j, w * W:(w + 1) * W], ps,
                                    AF.Identity, bias=bqk_sb[:, wi, j:j + 1])
                            else:
                                nc.vector.tensor_scalar_add(
                                    dst[:, j, w * W:(w + 1) * W], ps,
                                    bqk_sb[:, wi, j:j + 1])
                        return g

                    def v_group(it, sc):
                        def g():
                            ps = ppa.tile([P, W], f32, tag="pp", name="psvt")
                            for i in range(NCI):
                                mm(ps, xc[:, w, i, (it % 4) * P:
                                          (it % 4 + 1) * P],
                                   whv[:, i, :],
                                   start=(i == 0), stop=(i == NCI - 1))
                            vdst = (vp[:, it, :]
                                    .rearrange("p (h x) -> p h x",
                                               x=VS)[:, :, 0:HD])
                            psh = ps.rearrange("p (h x) -> p h x", x=HD)
                            if sc:
                                nc.scalar.copy(vdst, psh)
                            else:
                                nc.vector.tensor_copy(vdst, psh)
                        return g

                    ga = [qk_group(0, qT, 0, sc_a), qk_group(1, kT, 0, sc_a)]
                    ga += [v_group(it, sc_a) for it in range(4 * w, 4 * w + 4)]
                    gb = []
                    for j in range(1, CGP):
                        gb.append(qk_group(0, qT, j, sc_b))
                        gb.append(qk_group(1, kT, j, sc_b))
                    return ga, gb

                def oproj_groups(w, tail=False):
                    ws = slice(w * W, (w + 1) * W)

                    def o_group(m):
                        def g():
                            pool = pav if (tail and m % 2 == 1) else ppa
                            tag = "av" if pool is pav else "pp"
                            po = pool.tile([P, W], f32, tag=tag, name="po")
                            for i in range(CGP):
                                mm(po, wo_sb[:, i, m * P:(m + 1) * P],
                                   yT[:, i, ws],
                                   start=(i == 0), stop=(i == CGP - 1))
                            ot = otp.tile([P, W], f16, tag="ot")
                            if tail and m % 2 == 1:
                                nc.scalar.copy(ot, po)
                            else:
                                nc.vector.tensor_copy(ot, po)
                            dq = nc.scalar if (tail and m % 2 == 1) else nc.sync
                            dq.dma_start(
                                out=out_d.ap()[m * P:(m + 1) * P, ws],
                                in_=ot)
                        return g

                    return [o_group(m) for m in range(C // P)]

                # prologue: only what round (0,0) strictly needs
                ga0, gb0 = proj_groups(0, False, False)
                for g in ga0:
                    g()

                defer = gb0   # B-groups run inside their own window
                tail_work = None
                sc_flags = {1: (True, False), 2: (False, False),
                            3: (False, False)}
                for w in range(NW):
                    ws = slice(w * W, (w + 1) * W)
                    tq0 = w * W
                    ntk = 4 * (w + 1)

                    work = list(defer)
                    defer = []
                    if w == NW - 1:
                        # all out-projections land here: windows 0-2 are
                        # tensor-bound, window 3 is scalar-bound (exp), so
                        # oproj work fills w3's tensor idle.  Half of
                        # oproj(2) stays behind the last attention round
                        # to cover the final normalization chain (PE
                        # stays warm into oproj(3)).
                        og2 = oproj_groups(NW - 2, tail=True)
                        work += oproj_groups(0) + oproj_groups(1) + og2[:4]
                        tail_work = og2[4:]
                    if w < NW - 1:
                        ga, gb = proj_groups(w + 1, *sc_flags[w + 1])
                        work += ga
                        defer = gb
                    nwork = len(work)
                    total_tiles = CGP * ntk
                    seen = 0
                    emitted = 0

                    for j in range(CGP):
                        # round boundary: flush all uniformly-due work so
                        # chunked emission can never defer a q/k j-block
                        # past the round that reads it
                        while emitted < (nwork * seen) // total_tiles:
                            work[emitted]()
                            emitted += 1
                        ps_av = [pav.tile([VS, W], f32, tag="av",
                                          name=f"av{w}_{j}_{a}")
                                 for a in range(2)]
                        for p in range(ntk // 2):
                            # two tiles per batch: QK,QK then AV,AV,AV,AV.
                            # Full-row LDWEIGHTS hide inside same-shape mm
                            # chains but are exposed at every QK<->AV
                            # transition; pairing halves the transitions.
                            pair = (2 * p, 2 * p + 1)
                            c0s, pts = [], []
                            for i in pair:
                                nn = tq0 + W - max(tq0, i * P)
                                c0 = W - nn
                                c0s.append(c0)
                                psqk = pqk.tile([P, 2, W], f32, tag="qk")
                                for a in range(2):
                                    mm(psqk[:, a, c0:],
                                       kT[64 * a:64 * (a + 1), j,
                                          i * P:(i + 1) * P],
                                       qT[64 * a:64 * (a + 1), j,
                                          tq0 + c0:tq0 + W],
                                       start=True, stop=True)
                                seen += 1
                                # emit interleave work two groups at a
                                # time: halves the proj<->attention
                                # transitions (each pays an exposed
                                # LDWEIGHTS on the PE)
                                target = (nwork * seen) // total_tiles
                                if target - emitted >= 3 or seen == total_tiles:
                                    while emitted < target:
                                        work[emitted]()
                                        emitted += 1
                                pt_t = ptp.tile([P, 2, W], bf16, tag="pt")
                                pts.append(pt_t)
                                nc.scalar.activation(
                                    pt_t[:, :, c0:], psqk[:, :, c0:],
                                    AF.Exp, scale=escale)
                                if i * P >= tq0:  # diagonal: mask its block
                                    with tc.high_priority():
                                        nc.vector.tensor_mul(
                                            pt_t[:, :, c0:c0 + P],
                                            pt_t[:, :, c0:c0 + P], maskT2)
                            for ti, i in enumerate(pair):
                                c0 = c0s[ti]
                                for a in range(2):
                                    h = 2 * j + a
                                    mm(ps_av[a][:, c0:],
                                       vp[:, i, h * VS:(h + 1) * VS],
                                       pts[ti][:, a, c0:],
                                       start=(i == 0), stop=(i == ntk - 1))
                        for a in range(2):
                            av_sb = avsp.tile([VS, W], f32, tag="avs")
                            with tc.high_priority():
                                nc.vector.tensor_copy(av_sb, ps_av[a])
                            dn = dnp.tile([1, W], f32, tag="dn")
                            nc.vector.tensor_copy(dn, av_sb[HD:VS, :])
                            nc.vector.reciprocal_approx_fast(
                                out=dn, in_=dn)
                            rb = rbp.tile([HD, W], f32, tag="rb")
                            nc.gpsimd.partition_broadcast(rb, dn)
                            nc.vector.tensor_mul(
                                yT[64 * a:64 * (a + 1), j, ws],
                                av_sb[0:HD, :], rb)
                    while emitted < nwork:
                        work[emitted]()
                        emitted += 1

                for g in tail_work + oproj_groups(NW - 1, tail=True):
                    g()

    nc.compile()
    return nc


def get_nc():
    global _cached_nc
    if _cached_nc is None:
        _cached_nc = _build()
    return _cached_nc


def _swz(w, nci):
    """[K, N] -> contiguous [P, nci*N] in the 'p c n' SBUF layout."""
    k, n = w.shape
    return np.ascontiguousarray(
        w.reshape(nci, P, n).transpose(1, 0, 2).reshape(P, nci * n)
        .astype(ml_dtypes.bfloat16))


def _swzj(w):
    """[C, CG] -> contiguous [P, CGP*NCI*P] in the j-major SBUF layout."""
    return np.ascontiguousarray(
        w.reshape(NCI, P, CGP, P).transpose(1, 2, 0, 3)
        .reshape(P, CGP * NCI * P).astype(ml_dtypes.bfloat16))


def make_in_maps(x, Wq, bq, Wk, bk, Wv, bv, Wo, bo):
    x = np.asarray(x, np.float32)
    tri = np.triu(np.ones((P, P), np.float32))
    tri2 = np.concatenate([tri, tri], axis=1)
    in_maps = []
    for c in range(8):
        b, g = c // 2, c % 2
        cs = slice(g * CG, (g + 1) * CG)
        bqk = np.concatenate([
            np.asarray(bq, np.float32)[cs].reshape(CGP, P).T,
            np.asarray(bk, np.float32)[cs].reshape(CGP, P).T], axis=1)
        # x[b].T is [C, T]; window-major swizzle to [P, w, chunk, t]
        xt = x[b].T.astype(ml_dtypes.bfloat16)
        xt = (xt.reshape(NCI, P, NW, W).transpose(1, 2, 0, 3)
              .reshape(P, NW * NCI * W))
        in_maps.append({
            "xt": np.ascontiguousarray(xt),
            "wq": _swzj(np.asarray(Wq, np.float32)[:, cs]),
            "wk": _swzj(np.asarray(Wk, np.float32)[:, cs]),
            "wv": _swz(np.asarray(Wv, np.float32)[:, cs], NCI),
            "wo": _swz(np.asarray(Wo, np.float32)[cs, :], CGP),
            "bqk": np.ascontiguousarray(bqk),
            "maskT": tri2.astype(ml_dtypes.bfloat16),
        })
    return in_maps


def combine(results, Wv, bv, Wo, bo):
    const = (np.asarray(bv, np.float32) @ np.asarray(Wo, np.float32)
             + np.asarray(bo, np.float32))
    out = np.empty((B, T, C), np.float32)
    for b in range(B):
        acc = (results[2 * b]["outp"].astype(np.float32)
               + results[2 * b + 1]["outp"].astype(np.float32))
        out[b] = acc.T + const[None, :]
    return out


def kernel(x, Wq, bq, Wk, bk, Wv, bv, Wo, bo):
    nc = get_nc()
    in_maps = make_in_maps(x, Wq, bq, Wk, bk, Wv, bv, Wo, bo)
    res = run_bass_kernel_spmd(nc, in_maps, core_ids=list(range(8)))
    return combine(res.results, Wv, bv, Wo, bo)
